# revision 1
# baseline (speedup 1.0000x reference)
"""Bahdanau attention on 8 Trainium2 NeuronCores (Bass/Tile).

Reference computation (per batch b of 32, S=2048, D=1024, U=512):
    query  = dh[b] @ W1 + b1                      # [1, U]
    values = enc[b] @ W2 + b2                     # [S, U]
    scores = tanh(query + values) @ V + bv        # [S, 1]
    attn   = softmax(scores, axis=S)              # [S, 1]
    ctx    = sum_s attn[s] * enc[b, s]            # [D]

Sharding: data-parallel over batch, 4 batches per core, weights replicated.
All matmuls run in bf16 with fp32 PSUM accumulation (the reference regime).
Softmax skips the max-subtraction: |scores| <= sum|V| + |bv| (|tanh|<1), far
inside fp32 exp range.

Per-core layout (R = 4*2048 = 8192 rows):
  values^T[u, r] = sum_d W2[d, u] * enc^T[d, r]  -- PE, enc^T streamed from HBM
  t^T = tanh(values^T + query^T + b1 + b2)       -- ACT, per-partition bias
  score[r-tile, 1] = sum_u t^T[u, r-tile] * V    -- PE, accumulated into one
                                                    [128, 16] PSUM bank per batch
  w = exp(score + bv)                            -- one ACT Exp per batch
  denom = partition_all_reduce(row-sums)         -- DVE + GPSIMD
  attn = w * (1/denom)                           -- DVE
  ctx[1, d] = sum_s attn_bf16[s] * encN[s, d]    -- PE, enc natural layout
"""

import os
import sys

import numpy as np

for _p in ("/root/.axon_site/_ro/trn_rl_repo", "/opt/trn_rl_repo"):
    if _p not in sys.path and os.path.isdir(_p):
        sys.path.append(_p)

import ml_dtypes

B, S, D, U = 32, 2048, 1024, 512
N_CORES = 8
BP = B // N_CORES          # 4 batches per core
R = BP * S                 # 8192 rows per core
DT = D // 128              # 8 d-tiles
UT = U // 128              # 4 u-tiles
CHUNK = 512                # rows per values-matmul chunk
NCH = R // CHUNK           # 16 chunks per core (4 per batch)
ST = S // 128              # 16 s-tiles per batch

BF16 = ml_dtypes.bfloat16

_CACHE = {}


def build_program():
    """Build + compile the per-core Bass program (same program on all cores)."""
    if "nc" in _CACHE:
        return _CACHE["nc"]

    import concourse.tile as tile
    from concourse import bacc, bass, bass_isa, mybir
    from contextlib import ExitStack

    f32 = mybir.dt.float32
    bf = mybir.dt.bfloat16
    AF = mybir.ActivationFunctionType
    AX = mybir.AxisListType

    nc = bacc.Bacc("TRN2", target_bir_lowering=False, debug=False,
                   num_devices=N_CORES)

    encT = nc.dram_tensor("encT", [NCH, DT, 128, CHUNK], bf, kind="ExternalInput")
    encN = nc.dram_tensor("encN", [BP * ST, 128, D], bf, kind="ExternalInput")
    dhT = nc.dram_tensor("dhT", [DT, 128, BP], bf, kind="ExternalInput")
    W1d = nc.dram_tensor("W1t", [DT, 128, U], bf, kind="ExternalInput")
    W2d = nc.dram_tensor("W2t", [DT, 128, U], bf, kind="ExternalInput")
    Vd = nc.dram_tensor("Vt", [128, UT], bf, kind="ExternalInput")
    b12d = nc.dram_tensor("b12t", [128, UT], f32, kind="ExternalInput")
    bvd = nc.dram_tensor("bvbt", [128, 1], f32, kind="ExternalInput")
    attn = nc.dram_tensor("attn", [BP, 128, ST], f32, kind="ExternalOutput")
    ctxo = nc.dram_tensor("ctx", [BP, 1, D], f32, kind="ExternalOutput")

    with tile.TileContext(nc) as tc, ExitStack() as ex:
        const = ex.enter_context(tc.tile_pool(name="const", bufs=1))
        p_eT = ex.enter_context(tc.tile_pool(name="p_eT", bufs=3))
        p_eN = ex.enter_context(tc.tile_pool(name="p_eN", bufs=20))
        p_tt = ex.enter_context(tc.tile_pool(name="p_tt", bufs=8))
        p_sm = ex.enter_context(tc.tile_pool(name="p_sm", bufs=2))
        pv = ex.enter_context(tc.tile_pool(name="pv", bufs=4,
                                           space=bass.MemorySpace.PSUM))
        ps = ex.enter_context(tc.tile_pool(name="ps", bufs=2,
                                           space=bass.MemorySpace.PSUM))
        pc = ex.enter_context(tc.tile_pool(name="pc", bufs=2,
                                           space=bass.MemorySpace.PSUM))

        W2_sb = const.tile([128, DT, U], bf)
        W1_sb = const.tile([128, DT, U], bf)
        dh_sb = const.tile([128, DT, BP], bf)
        for d in range(DT):
            nc.sync.dma_start(W2_sb[:, d, :], W2d[d])
            nc.sync.dma_start(W1_sb[:, d, :], W1d[d])
            nc.sync.dma_start(dh_sb[:, d, :], dhT[d])
        V_sb = const.tile([128, UT], bf)
        nc.sync.dma_start(V_sb[:], Vd.ap())
        b12_sb = const.tile([128, UT], f32)
        nc.sync.dma_start(b12_sb[:], b12d.ap())
        bv_sb = const.tile([128, 1], f32)
        nc.sync.dma_start(bv_sb[:], bvd.ap())

        # query^T[u, b] + b1 + b2, laid out [128, ut, b]
        qb_sb = const.tile([128, UT, BP], f32)
        for ut in range(UT):
            qT = ps.tile([128, BP], f32, tag="s")
            for d in range(DT):
                nc.tensor.matmul(qT[:], W1_sb[:, d, ut * 128:(ut + 1) * 128],
                                 dh_sb[:, d, :], start=(d == 0), stop=(d == DT - 1))
            nc.vector.tensor_scalar_add(qb_sb[:, ut, :], qT[:],
                                        b12_sb[:, ut:ut + 1])

        for b in range(BP):
            w_sc = ps.tile([128, ST], f32, tag="s")     # scores, one PSUM bank
            eNs = []
            for cc in range(NCH // BP):                 # 4 chunks per batch
                c = b * (NCH // BP) + cc
                eT = p_eT.tile([128, DT, CHUNK], bf, tag="eT")
                for d in range(DT):
                    nc.sync.dma_start(eT[:, d, :], encT[c, d])
                # prefetch this chunk's share of the natural-layout tiles
                for k in range(4):
                    st = cc * 4 + k
                    eN = p_eN.tile([128, D], bf, tag="eN")
                    nc.sync.dma_start(eN[:], encN[b * ST + st])
                    eNs.append(eN)
                tts = []
                for ut in range(UT):
                    v = pv.tile([128, CHUNK], f32, tag="v")
                    for d in range(DT):
                        nc.tensor.matmul(v[:], W2_sb[:, d, ut * 128:(ut + 1) * 128],
                                         eT[:, d, :], start=(d == 0),
                                         stop=(d == DT - 1))
                    t_ = p_tt.tile([128, CHUNK], bf, tag="tt")
                    nc.scalar.activation(t_[:], v[:], AF.Tanh,
                                         bias=qb_sb[:, ut, b:b + 1])
                    tts.append(t_)
                for rt in range(CHUNK // 128):
                    j = cc * 4 + rt
                    for ut in range(UT):
                        nc.tensor.matmul(w_sc[:, j:j + 1],
                                         tts[ut][:, rt * 128:(rt + 1) * 128],
                                         V_sb[:, ut:ut + 1],
                                         start=(ut == 0), stop=(ut == UT - 1))
            # ---- batch-level epilogue ----
            w_b = p_sm.tile([128, ST], f32, tag="w")
            nc.scalar.activation(w_b[:], w_sc[:], AF.Exp, bias=bv_sb[:])
            rows = p_sm.tile([128, 1], f32, tag="rows")
            nc.vector.reduce_sum(rows[:], w_b[:], axis=AX.X)
            asum = p_sm.tile([128, 1], f32, tag="asum")
            nc.gpsimd.partition_all_reduce(asum[:], rows[:], 128,
                                           bass_isa.ReduceOp.add)
            rec = p_sm.tile([128, 1], f32, tag="rec")
            nc.vector.reciprocal(rec[:], asum[:])
            attn_sb = p_sm.tile([128, ST], f32, tag="attn")
            nc.vector.tensor_scalar_mul(attn_sb[:], w_b[:], rec[:])
            nc.sync.dma_start(attn[b], attn_sb[:])
            wn = p_sm.tile([128, ST], bf, tag="wn")
            nc.vector.tensor_scalar_mul(wn[:], w_b[:], rec[:])
            c0 = pc.tile([1, 512], f32, tag="ctx")
            c1 = pc.tile([1, 512], f32, tag="ctx")
            for st in range(ST):
                nc.tensor.matmul(c0[:], wn[:, st:st + 1], eNs[st][:, 0:512],
                                 start=(st == 0), stop=(st == ST - 1))
                nc.tensor.matmul(c1[:], wn[:, st:st + 1], eNs[st][:, 512:1024],
                                 start=(st == 0), stop=(st == ST - 1))
            ctx_sb = p_sm.tile([1, D], f32, tag="ctxsb")
            nc.vector.tensor_copy(ctx_sb[:, 0:512], c0[:])
            nc.vector.tensor_copy(ctx_sb[:, 512:1024], c1[:])
            nc.sync.dma_start(ctxo[b], ctx_sb[:])

    nc.compile()
    nc.finalize()
    _CACHE["nc"] = nc
    return nc


def prep_in_maps(decoder_hidden, encoder_outputs, W1, b1, W2, b2, V, bv):
    """Host-side shard/cast/transpose -> per-core input maps."""
    dh = np.asarray(decoder_hidden, np.float32).reshape(B, D)
    enc = np.asarray(encoder_outputs, np.float32)
    W1 = np.asarray(W1, np.float32)
    b1 = np.asarray(b1, np.float32)
    W2 = np.asarray(W2, np.float32)
    b2 = np.asarray(b2, np.float32)
    V = np.asarray(V, np.float32).reshape(U)
    bv = np.asarray(bv, np.float32).reshape(1)

    W1_t = np.ascontiguousarray(W1.reshape(DT, 128, U)).astype(BF16)
    W2_t = np.ascontiguousarray(W2.reshape(DT, 128, U)).astype(BF16)
    V_t = np.ascontiguousarray(V.reshape(UT, 128).T).astype(BF16)
    b12_t = np.ascontiguousarray((b1 + b2).reshape(UT, 128).T).astype(np.float32)
    bv_t = np.full((128, 1), float(bv[0]), np.float32)

    in_maps = []
    for c in range(N_CORES):
        encs = enc[c * BP:(c + 1) * BP].reshape(R, D)
        encT = np.ascontiguousarray(
            encs.T.reshape(DT, 128, NCH, CHUNK).transpose(2, 0, 1, 3)
        ).astype(BF16)
        encN = encs.reshape(BP * ST, 128, D).astype(BF16)
        dhT = np.ascontiguousarray(
            dh[c * BP:(c + 1) * BP].T.reshape(DT, 128, BP)
        ).astype(BF16)
        in_maps.append({
            "encT": encT, "encN": encN, "dhT": dhT,
            "W1t": W1_t, "W2t": W2_t, "Vt": V_t,
            "b12t": b12_t, "bvbt": bv_t,
        })
    return in_maps


def gather_outputs(results):
    """Per-core {attn:[BP,128,ST], ctx:[BP,1,D]} -> full fp32 outputs."""
    attn = np.concatenate(
        [r["attn"].transpose(0, 2, 1).reshape(BP, S, 1) for r in results], axis=0
    ).astype(np.float32)
    ctx = np.concatenate(
        [r["ctx"].reshape(BP, D) for r in results], axis=0
    ).astype(np.float32)
    return ctx, attn


def kernel(decoder_hidden, encoder_outputs, W1, b1, W2, b2, V, bv):
    from concourse.bass_utils import run_bass_kernel_spmd

    nc = build_program()
    in_maps = prep_in_maps(decoder_hidden, encoder_outputs, W1, b1, W2, b2, V, bv)
    res = run_bass_kernel_spmd(nc, in_maps, list(range(N_CORES)))
    _CACHE["last_results"] = res
    return gather_outputs(res.results)


# revision 45
# speedup vs baseline: 136.6431x; 136.6431x over previous
"""Bahdanau attention on 8 Trainium2 NeuronCores (Bass/Tile).

Reference computation (per batch b of 32, S=2048, D=1024, U=512):
    query  = dh[b] @ W1 + b1                      # [1, U]
    values = enc[b] @ W2 + b2                     # [S, U]
    scores = tanh(query + values) @ V + bv        # [S, 1]
    attn   = softmax(scores, axis=S)              # [S, 1]
    ctx    = sum_s attn[s] * enc[b, s]            # [D]

Sharding: data-parallel over batch, 4 batches per core, weights replicated.
Matmuls run in bf16 with fp32 PSUM accumulation (the reference regime).
Softmax skips the max-subtraction: |scores| <= sum|V| + |bv| (|tanh|<1), far
inside fp32 exp range.

Per-core dataflow (R = 4*2048 = 8192 rows). Only ONE 16MB HBM stream (enc^T):
  values^T[u, r] = sum_d W2[d, u] * enc^T[d, r]   -- PE
  t^T = tanh(values^T + query^T + b1 + b2)        -- ACT, per-partition bias
  score^T[1, r] = sum_u V[u] * t^T[u, r]          -- PE (lhsT = V)
  w[1, r] = exp(score + bv); denom = sum(w)       -- ACT + DVE (single row)
  attn = w / denom                                -- DVE
  wbc[128, r] = broadcast(attn_bf16)              -- GPSIMD
  ctx^T[d_tile, 1] = reduce_r(enc^T * wbc)        -- DVE tensor_tensor_reduce
No natural-layout enc needed -> no second HBM stream, no PE context matmuls.
"""

import os
import sys

import numpy as np

for _p in ("/root/.axon_site/_ro/trn_rl_repo", "/opt/trn_rl_repo"):
    if _p not in sys.path and os.path.isdir(_p):
        sys.path.append(_p)

import ml_dtypes

B, S, D, U = 32, 2048, 1024, 512
N_CORES = 8
BP = B // N_CORES          # 4 batches per core
R = BP * S                 # 8192 rows per core
DT = D // 128              # 8 d-tiles
UT = U // 128              # 4 u-tiles
CHUNK = 512                # rows per values-matmul chunk
NCH = R // CHUNK           # 16 chunks per core (4 per batch)
CPB = NCH // BP            # 4 chunks per batch

BF16 = ml_dtypes.bfloat16

_CACHE = {}


def build_program(n_iters=1, variant="full"):
    """Build + compile the per-core Bass program (same program on all cores).

    n_iters > 1 wraps the whole body in a device-side For_i loop — used only
    for wall-clock timing (per-iteration = total / n_iters); kernel() uses 1.
    variant: "full" | "noctx" (skip context pass) | "vals" (values+tanh only)
             | "dma" (input DMA only) — perf-isolation builds for timing.
    """
    key = ("nc", n_iters, variant)
    if key in _CACHE:
        return _CACHE[key]

    import concourse.tile as tile
    from concourse import bacc, bass, bass_isa, mybir
    from contextlib import ExitStack

    f32 = mybir.dt.float32
    bf = mybir.dt.bfloat16
    AF = mybir.ActivationFunctionType
    AX = mybir.AxisListType
    MUL = mybir.AluOpType.mult
    ADD = mybir.AluOpType.add

    nc = bacc.Bacc("TRN2", target_bir_lowering=False, debug=False,
                   num_devices=N_CORES)

    encT = nc.dram_tensor("encT", [NCH, 128, DT * CHUNK], bf, kind="ExternalInput")
    dhT = nc.dram_tensor("dhT", [128, DT * BP], bf, kind="ExternalInput")
    W1d = nc.dram_tensor("W1t", [128, DT * U], bf, kind="ExternalInput")
    W2d = nc.dram_tensor("W2t", [128, DT * U], bf, kind="ExternalInput")
    Vd = nc.dram_tensor("Vt", [128, UT], bf, kind="ExternalInput")
    b12d = nc.dram_tensor("b12t", [128, UT], f32, kind="ExternalInput")
    bvd = nc.dram_tensor("bvbt", [128, 1], f32, kind="ExternalInput")
    attn = nc.dram_tensor("attn", [BP, 1, S], f32, kind="ExternalOutput")
    ctxo = nc.dram_tensor("ctx", [BP, 128, DT], f32, kind="ExternalOutput")

    with tile.TileContext(nc) as tc, ExitStack() as ex:
        if n_iters > 1:
            ex.enter_context(tc.For_i(
                0, n_iters, 1,
                hint_engines=(mybir.EngineType.PE, mybir.EngineType.Activation,
                              mybir.EngineType.DVE, mybir.EngineType.SP,
                              mybir.EngineType.Pool),
            ))

        const = ex.enter_context(tc.tile_pool(name="const", bufs=1))
        p_eT = ex.enter_context(tc.tile_pool(name="p_eT", bufs=10))
        p_tt = ex.enter_context(tc.tile_pool(name="p_tt", bufs=12))
        p_scr = ex.enter_context(tc.tile_pool(name="p_scr", bufs=2))
        p_wb = ex.enter_context(tc.tile_pool(name="p_wb", bufs=2))
        p_sm = ex.enter_context(tc.tile_pool(name="p_sm", bufs=2))
        pv = ex.enter_context(tc.tile_pool(name="pv", bufs=4,
                                           space=bass.MemorySpace.PSUM))
        ps = ex.enter_context(tc.tile_pool(name="ps", bufs=4,
                                           space=bass.MemorySpace.PSUM))

        W2_sb = const.tile([128, DT, U], bf)
        W1_sb = const.tile([128, DT, U], bf)
        dh_sb = const.tile([128, DT, BP], bf)
        nc.sync.dma_start(W2_sb[:], W2d.ap())
        nc.sync.dma_start(W1_sb[:], W1d.ap())
        nc.sync.dma_start(dh_sb[:], dhT.ap())
        V_sb = const.tile([128, UT], bf)
        nc.sync.dma_start(V_sb[:], Vd.ap())
        b12_sb = const.tile([128, UT], f32)
        nc.sync.dma_start(b12_sb[:], b12d.ap())
        bv_sb = const.tile([128, 1], f32)
        nc.sync.dma_start(bv_sb[:], bvd.ap())
        ones_sb = const.tile([1, 128], bf)
        nc.vector.memset(ones_sb[:], 1.0)

        # query^T[u, b] + b1 + b2, laid out [128, ut, b]
        qb_sb = const.tile([128, UT, BP], f32)
        for ut in range(UT):
            qT = ps.tile([128, BP], f32, tag="s")
            for d in range(DT):
                nc.tensor.matmul(qT[:], W1_sb[:, d, ut * 128:(ut + 1) * 128],
                                 dh_sb[:, d, :], start=(d == 0), stop=(d == DT - 1))
            nc.vector.tensor_scalar_add(qb_sb[:, ut, :], qT[:],
                                        b12_sb[:, ut:ut + 1])

        want_ctx = variant == "full"
        want_scores = want_ctx or variant == "noctx"
        want_vals = want_scores or variant == "vals"

        if variant.startswith("pe"):
            # pure PE/ACT pipeline: one resident chunk, 16x the matmul work
            # "pe"    : tanh consumer (like real kernel)
            # "pent"  : tiny DVE consumer instead of tanh (test ACT coupling)
            # "pe1"   : one giant accumulation group per psum tile (test
            #           group-boundary/bank-cycling overhead)
            eT = p_eT.tile([128, DT, CHUNK], bf, tag="eT")
            nc.sync.dma_start(eT[:], encT[0])
            token = p_sm.tile([1, 4], f32, tag="token")
            if variant == "pew":
                # all MMs share ONE stationary tile: does weight reuse help?
                for c in range(NCH):
                    for ut in range(UT):
                        v = pv.tile([128, CHUNK], f32, tag="v")
                        for d in range(DT):
                            nc.tensor.matmul(v[:], W2_sb[:, 0, 0:128],
                                             eT[:, d, :], start=(d == 0),
                                             stop=(d == DT - 1))
                        nc.vector.tensor_copy(token[:], v[0:1, 0:4])
            elif variant == "pn1024":
                # N=1024 stream, psum tile spanning 2 banks (half the MM count;
                # data content irrelevant — throughput microbench)
                eT2 = p_eT.tile([128, DT, 2 * CHUNK], bf, tag="eT2")
                nc.sync.dma_start(eT2[:, :, 0:CHUNK], encT[0])
                nc.sync.dma_start(eT2[:, :, CHUNK:2 * CHUNK], encT[1])
                pv2 = ex.enter_context(
                    tc.tile_pool(name="pv2", bufs=3, space=bass.MemorySpace.PSUM))
                for c in range(NCH // 2):
                    for ut in range(UT):
                        v = pv2.tile([128, 2 * CHUNK], f32, tag="v2")
                        for d in range(DT):
                            nc.tensor.matmul(
                                v[:], W2_sb[:, d, ut * 128:(ut + 1) * 128],
                                eT2[:, d, :], start=(d == 0), stop=(d == DT - 1))
                        nc.vector.tensor_copy(token[:], v[0:1, 0:4])
            elif variant == "pe1":
                for c in range(NCH):
                    v = pv.tile([128, CHUNK], f32, tag="v")
                    n_mm = UT * DT
                    for i in range(n_mm):
                        ut, d = divmod(i, DT)
                        nc.tensor.matmul(v[:], W2_sb[:, d, ut * 128:(ut + 1) * 128],
                                         eT[:, d, :], start=(i == 0),
                                         stop=(i == n_mm - 1))
                    nc.vector.tensor_copy(token[:], v[0:1, 0:4])
            else:
                for c in range(NCH):
                    for ut in range(UT):
                        v = pv.tile([128, CHUNK], f32, tag="v")
                        for d in range(DT):
                            nc.tensor.matmul(v[:],
                                             W2_sb[:, d, ut * 128:(ut + 1) * 128],
                                             eT[:, d, :], start=(d == 0),
                                             stop=(d == DT - 1))
                        if variant == "pent":
                            nc.vector.tensor_copy(token[:], v[0:1, 0:4])
                        else:
                            t_ = p_tt.tile([128, CHUNK], bf, tag="tt")
                            nc.scalar.activation(t_[:], v[:], AF.Tanh,
                                                 bias=qb_sb[:, ut, 0:1])
                            nc.vector.tensor_copy(token[:], t_[0:1, 0:4])
            nc.sync.dma_start(ctxo[0][0:1, 0:4], token[:])

        for b in range(0 if variant.startswith("pe") else BP):
            score_flat = p_sm.tile([1, S], f32, tag="sf")
            token = p_sm.tile([1, DT], f32, tag="token")
            eTs = []
            for cc in range(CPB):
                c = b * CPB + cc
                eT = p_eT.tile([128, DT, CHUNK], bf, tag="eT")
                eng = nc.sync if (c % 2 == 0) else nc.scalar
                eng.dma_start(eT[:], encT[c])
                eTs.append(eT)
                if variant == "dma":
                    # consume the tile so the DMA can't be dead-code'd
                    nc.vector.tensor_copy(token[:], eT[0:1, :, 0])
                    continue
                tts = []
                for ut in range(UT):
                    v = pv.tile([128, CHUNK], f32, tag="v")
                    for d in range(DT):
                        nc.tensor.matmul(v[:], W2_sb[:, d, ut * 128:(ut + 1) * 128],
                                         eT[:, d, :], start=(d == 0),
                                         stop=(d == DT - 1))
                    t_ = p_tt.tile([128, CHUNK], bf, tag="tt")
                    nc.scalar.activation(t_[:], v[:], AF.Tanh,
                                         bias=qb_sb[:, ut, b:b + 1])
                    tts.append(t_)
                if not want_scores:
                    for ut in range(UT):
                        nc.vector.tensor_copy(token[:, 0:4], tts[ut][0:1, 0:4])
                    continue
                # score row for this chunk: [1, 512] = sum_u V[u] * t^T[u, :]
                s_row = ps.tile([1, CHUNK], f32, tag="s")
                for ut in range(UT):
                    nc.tensor.matmul(s_row[:], V_sb[:, ut:ut + 1], tts[ut][:],
                                     start=(ut == 0), stop=(ut == UT - 1))
                # copy on ACT (Copy lives in every act table; keeps the strict-
                # FIFO DVE queue out of the PE-gating path)
                nc.scalar.activation(
                    score_flat[:, cc * CHUNK:(cc + 1) * CHUNK], s_row[:],
                    AF.Copy)
            if not want_scores:
                nc.sync.dma_start(ctxo[b][0:1, 0:DT], token[:])
                continue
            # ---- batch-level softmax (single row) ----
            w_flat = p_sm.tile([1, S], f32, tag="w")
            nc.scalar.activation(w_flat[:], score_flat[:], AF.Exp,
                                 bias=bv_sb[0:1, :])
            den = p_sm.tile([1, 1], f32, tag="den")
            nc.vector.reduce_sum(den[:], w_flat[:], axis=AX.X)
            rec = p_sm.tile([1, 1], f32, tag="rec")
            nc.vector.reciprocal(rec[:], den[:])
            attn_row = p_sm.tile([1, S], f32, tag="attn")
            nc.vector.tensor_scalar_mul(attn_row[:], w_flat[:], rec[:])
            nc.sync.dma_start(attn[b], attn_row[:])
            if not want_ctx:
                continue
            # ---- context via DVE reduce over rows ----
            wn_flat = p_sm.tile([1, S], bf, tag="wn")
            nc.vector.tensor_scalar_mul(wn_flat[:], w_flat[:], rec[:])
            wbc = p_wb.tile([128, S], bf, tag="wbc")
            nc.gpsimd.partition_broadcast(wbc[:], wn_flat[:], 128)
            acc = p_sm.tile([128, DT, CPB], f32, tag="acc")
            for cc in range(CPB):
                scr = p_scr.tile([128, DT, CHUNK], bf, tag="scr")
                w_sl = (wbc[:, cc * CHUNK:(cc + 1) * CHUNK]
                        .unsqueeze(1).broadcast_to((128, DT, CHUNK)))
                nc.vector.tensor_mul(scr[:], eTs[cc][:], w_sl)
                nc.vector.reduce_sum(acc[:, :, cc:cc + 1], scr[:], axis=AX.X)
            ctxt = p_sm.tile([128, DT], f32, tag="ctxt")
            nc.vector.reduce_sum(ctxt[:], acc[:], axis=AX.X)
            nc.sync.dma_start(ctxo[b], ctxt[:])

    nc.compile()
    nc.finalize()
    _CACHE[key] = nc
    return nc


def prep_in_maps(decoder_hidden, encoder_outputs, W1, b1, W2, b2, V, bv):
    """Host-side shard/cast/transpose -> per-core input maps."""
    dh = np.asarray(decoder_hidden, np.float32).reshape(B, D)
    enc = np.asarray(encoder_outputs, np.float32)
    W1 = np.asarray(W1, np.float32)
    b1 = np.asarray(b1, np.float32)
    W2 = np.asarray(W2, np.float32)
    b2 = np.asarray(b2, np.float32)
    V = np.asarray(V, np.float32).reshape(U)
    bv = np.asarray(bv, np.float32).reshape(1)

    W1_t = np.ascontiguousarray(
        W1.reshape(DT, 128, U).transpose(1, 0, 2).reshape(128, DT * U)
    ).astype(BF16)
    W2_t = np.ascontiguousarray(
        W2.reshape(DT, 128, U).transpose(1, 0, 2).reshape(128, DT * U)
    ).astype(BF16)
    V_t = np.ascontiguousarray(V.reshape(UT, 128).T).astype(BF16)
    b12_t = np.ascontiguousarray((b1 + b2).reshape(UT, 128).T).astype(np.float32)
    bv_t = np.full((128, 1), float(bv[0]), np.float32)

    in_maps = []
    for c in range(N_CORES):
        encs = enc[c * BP:(c + 1) * BP].reshape(R, D)
        encT = np.ascontiguousarray(
            encs.T.reshape(DT, 128, NCH, CHUNK).transpose(2, 1, 0, 3)
            .reshape(NCH, 128, DT * CHUNK)
        ).astype(BF16)
        dhT = np.ascontiguousarray(
            dh[c * BP:(c + 1) * BP].T.reshape(DT, 128, BP).transpose(1, 0, 2)
            .reshape(128, DT * BP)
        ).astype(BF16)
        in_maps.append({
            "encT": encT, "dhT": dhT,
            "W1t": W1_t, "W2t": W2_t, "Vt": V_t,
            "b12t": b12_t, "bvbt": bv_t,
        })
    return in_maps


def gather_outputs(results):
    """Per-core {attn:[BP,1,S], ctx:[BP,128,DT]} -> full fp32 outputs."""
    attn = np.concatenate(
        [r["attn"].reshape(BP, S, 1) for r in results], axis=0
    ).astype(np.float32)
    ctx = np.concatenate(
        [r["ctx"].transpose(0, 2, 1).reshape(BP, D) for r in results], axis=0
    ).astype(np.float32)
    return ctx, attn


def kernel(decoder_hidden, encoder_outputs, W1, b1, W2, b2, V, bv):
    from concourse.bass_utils import run_bass_kernel_spmd

    nc = build_program()
    in_maps = prep_in_maps(decoder_hidden, encoder_outputs, W1, b1, W2, b2, V, bv)
    res = run_bass_kernel_spmd(nc, in_maps, list(range(N_CORES)))
    _CACHE["last_results"] = res
    return gather_outputs(res.results)
